# revision 20
# baseline (speedup 1.0000x reference)
"""Multi-head attention (B=2, S=2048, D=1024, H=16, Dk=64) on 8 TRN2 cores.

Sharding: core c handles batch b=c//4 and head group g=c%4 (heads 4g..4g+3,
i.e. projection output dims 256g..256g+256). Fully independent cores, no
collectives.

v8 (from v3 via trace-driven iterations; traced exec 170.8us -> 128.2us):
  - DMA issue-rate fix: v3 issued 129 small dma_starts (~700ns of Sync
    engine time each), stretching the 13.6MB input load to ~90us and
    starving both compute engines early. v8 packs x host-side as
    [128, NKC*S] (the exact SBUF image) and issues ~21 large transfers.
  - DMA priority: the DGE ring fair-shares bandwidth across its 8
    outstanding-transfer slots, so the critical prefix (wk/wq m0 + K/Q
    chunk 0) is split 3-ways per chunk to monopolize all 8 slots and
    land ~8us earlier; bqk/vb/cmw ride the gpsimd ring. xv chunk c
    must land before proj_v(c)'s chain position or the PE FIFO stalls.
  - PE warmup (28 matmuls) + dummy exp read a memset scratch tile, so
    the HAM clock ramp and ACT table load start immediately at +7.5us
    with no DMA dependency.
  - PV(0)/PV(1) are deferred into the next score-chunk's window (run
    from saved exp tiles via the pproj PSUM pool), removing the
    17us/6us ACT holes at chunk boundaries where v3 drained the next
    K/Q projection inline; PV(2)/PV(3) stay inline (ppv pool, lag-2).
  - Filler chain [kq0, kq1, v0, pv0*, kq2, v2, v1, pv1*, kq3, v3]
    (v2 before sc2's inline-PV v-drain) with per-(sc,p) pull rates;
    hard drains give correctness, rates only shape the overlap.
  - Schedule theory: PE streamed-column work ~87us (score b2-pairs run
    row-group-concurrent), ACT exp ~81us, so the kernel is PE-bound;
    the schedule keeps the PE dense (3.9us total idle) and lets ACT
    idle early rather than late.

Device pipeline per core (all matmul inputs bf16, fp32 PSUM accumulation):
  - K/Q projections into transposed layout QT/KT [256 dims, 2048 seq] per
    512-seq chunk (lhsT = W^T chunk, rhs = x^T chunk); per-partition bias
    added on DVE during the PSUM->SBUF copy; Wq pre-scaled by 1/8.
  - V projection into natural layout VH [seq, dims] with per-head 65 cols
    (col 64 = ones column giving the softmax denominator).
  - Scores transposed: S^T[kv,q] = KT-block (stationary) @ QT-chunk, a head
    pair sharing one [128,1024] PSUM tile (the two 64-row stationaries run
    row-group-parallel on the PE). Causal: upper blocks skipped; diagonal
    blocks N-trimmed, masked with a [128,128] tril-window mul on DVE, exp
    AP trimmed to match. No max-subtraction (|scores| <= ~4).
  - PV: O'^T[65, q] += VH-block (stationary) @ E-block, accumulated over
    kv blocks in PSUM. Row 64 = sum(E). DVE copy out, DMA out on
    alternating rings; final division + head interleave on host.
"""

import numpy as np
import ml_dtypes

B, S, D, H, DK = 2, 2048, 1024, 16, 64
N_CORES = 8
HPC = 4          # heads per core
GD = HPC * DK    # group dims = 256
W65 = HPC * 65   # V-projection output cols (64 data + 1 ones per head)
QC = 512         # q-chunk (also seq projection chunk)
N_QC = S // QC   # 4
N_KB = S // 128  # 16
NKC = D // 128   # 8 contraction chunks
bf16 = ml_dtypes.bfloat16

_cache: dict = {}


def _build(mode: str):
    """mode: 'causal' (diag-window masks, upper blocks skipped),
    'none' (no masking, all blocks), 'general' (per-block masks from DRAM)."""
    import concourse.bass as bass
    import concourse.mybir as mybir
    from concourse import bacc
    from concourse.tile import TileContext

    fp32 = mybir.dt.float32
    bf = mybir.dt.bfloat16
    AF = mybir.ActivationFunctionType

    nc = bacc.Bacc("TRN2", target_bir_lowering=False, debug=False,
                   num_devices=N_CORES)

    # host-prepacked inputs (see kernel() below); x tensors are packed as
    # the exact SBUF image [128, kc-major * seq] so chunked column loads
    # use identical APs on both sides.
    xq = nc.dram_tensor("xq", [128, NKC * S], bf, kind="ExternalInput")
    xk = nc.dram_tensor("xk", [128, NKC * S], bf, kind="ExternalInput")
    xv = nc.dram_tensor("xv", [128, NKC * S], bf, kind="ExternalInput")
    wq = nc.dram_tensor("wq", [128, NKC * GD], bf, kind="ExternalInput")
    wk = nc.dram_tensor("wk", [128, NKC * GD], bf, kind="ExternalInput")
    wv = nc.dram_tensor("wv", [128, NKC * W65], bf, kind="ExternalInput")
    vb = nc.dram_tensor("vb", [128, W65], bf, kind="ExternalInput")
    bqk = nc.dram_tensor("bqk", [128, 4], fp32, kind="ExternalInput")
    if mode == "causal":
        cmw = nc.dram_tensor("cmw", [128, 128], bf, kind="ExternalInput")
    elif mode == "general":
        amaskT = nc.dram_tensor("amaskT", [S, S], bf, kind="ExternalInput")
    out = nc.dram_tensor("out", [HPC, 65, S], bf, kind="ExternalOutput")

    with TileContext(nc) as tc:
        with (
            tc.tile_pool(name="res", bufs=1) as res,
            tc.tile_pool(name="mload", bufs=4) as mload,
            tc.tile_pool(name="eload", bufs=28) as eload,
            tc.tile_pool(name="oout", bufs=6) as oout,
            tc.tile_pool(name="pproj", bufs=2, space="PSUM") as pproj,
            tc.tile_pool(name="pscore", bufs=2, space="PSUM") as pscore,
            tc.tile_pool(name="ppv", bufs=2, space="PSUM") as ppv,
        ):
            # ---- resident tiles ----
            xk_s = res.tile([128, NKC * S], bf, tag="xk")
            xq_s = res.tile([128, NKC * S], bf, tag="xq")
            xv_s = res.tile([128, NKC * S], bf, tag="xv")
            wq_s = res.tile([128, NKC * GD], bf, tag="wq")
            wk_s = res.tile([128, NKC * GD], bf, tag="wk")
            wv_s = res.tile([128, NKC * W65], bf, tag="wv")
            vb_s = res.tile([128, W65], bf, tag="vb")
            bqk_s = res.tile([128, 4], fp32, tag="bqk")

            # scratch tiles for DMA-independent warmup work
            scw = res.tile([128, 256], bf, tag="scw")
            sce = res.tile([128, 4], fp32, tag="sce")
            dexp = res.tile([128, 4], bf, tag="dexp")
            nc.gpsimd.memset(scw[:], 0)
            nc.gpsimd.memset(sce[:], 0.0)

            # dummy exp: pulls the ~2.7us ACT_TABLE_LOAD+drain to kernel
            # start instead of paying it right before the first real exp.
            nc.scalar.activation(dexp[:], sce[:], AF.Exp)

            # PE warmup during the DMA head: dummy matmuls on the memset
            # scratch tile bring the HAM clock gate to full rate before
            # the first real projection (no DMA dependency).
            for _ in range(28):
                wps = pproj.tile([128, 256], fp32, tag="proj")
                nc.tensor.matmul(wps[:], scw[:, 0:128], scw[:, 0:256],
                                 start=True, stop=True)

            # ---- DMA schedule: few big transfers, priority order ----
            m0 = NKC * 128

            def xchunk(dst, src, c, split=1):
                d3 = dst[:].rearrange("p (k s) -> p k s", k=NKC)
                s3 = src[:, :].rearrange("p (k s) -> p k s", k=NKC)
                bounds = {1: ((0, 8),), 3: ((0, 3), (3, 6), (6, 8))}[split]
                for k0, k1 in bounds:
                    nc.sync.dma_start(d3[:, k0:k1, c * QC:(c + 1) * QC],
                                      s3[:, k0:k1, c * QC:(c + 1) * QC])

            nc.sync.dma_start(wk_s[:, 0:m0], wk[:, 0:m0])
            xchunk(xk_s, xk, 0, split=3)
            nc.sync.dma_start(wq_s[:, 0:m0], wq[:, 0:m0])
            xchunk(xq_s, xq, 0, split=3)
            nc.sync.dma_start(wk_s[:, m0:2 * m0], wk[:, m0:2 * m0])
            nc.sync.dma_start(wq_s[:, m0:2 * m0], wq[:, m0:2 * m0])
            xchunk(xk_s, xk, 1)
            xchunk(xq_s, xq, 1)
            xchunk(xv_s, xv, 0)
            nc.sync.dma_start(wv_s[:], wv[:, :])
            xchunk(xv_s, xv, 1)
            xchunk(xk_s, xk, 2)
            xchunk(xq_s, xq, 2)
            xchunk(xv_s, xv, 2)
            xchunk(xk_s, xk, 3)
            xchunk(xq_s, xq, 3)
            xchunk(xv_s, xv, 3)
            nc.gpsimd.dma_start(bqk_s[:], bqk[:, :])
            nc.gpsimd.dma_start(vb_s[:], vb[:, :])
            if mode == "causal":
                cmw_s = res.tile([128, 128], bf, tag="cmw")
                nc.gpsimd.dma_start(cmw_s[:], cmw[:, :])

            # resident projected activations
            qt_s = [res.tile([128, S], bf, tag=f"qt{m}", name=f"qt{m}")
                    for m in range(2)]
            kt_s = [res.tile([128, S], bf, tag=f"kt{m}", name=f"kt{m}")
                    for m in range(2)]
            vh_s = res.tile([128, N_KB * W65], bf, tag="vh")

            # ---- projection generators (filler work) ----
            def proj_kq_chunk(c):
                # m-major so heads 0/1 (m=0) are ready before heads 2/3
                for m in range(2):
                    for w_s, x_s, dst, bcol in ((wk_s, xk_s, kt_s, 2),
                                                (wq_s, xq_s, qt_s, 0)):
                        ps = pproj.tile([128, QC], fp32, tag="proj")
                        for kc in range(NKC):
                            wsl = w_s[:, m * NKC * 128 + kc * 128:
                                      m * NKC * 128 + (kc + 1) * 128]
                            nc.tensor.matmul(
                                ps[:], wsl,
                                x_s[:, kc * S + c * QC: kc * S + (c + 1) * QC],
                                start=(kc == 0), stop=(kc == NKC - 1))
                            yield None
                        bias = bqk_s[:, bcol + m:bcol + m + 1]
                        nc.vector.tensor_scalar_add(
                            dst[m][:, c * QC:(c + 1) * QC], ps[:], bias)
                        yield None
                    if m == 0:
                        yield f"kq{c}m0"
                yield f"kq{c}"

            def proj_v(c):
                for sb in range(c * 4, (c + 1) * 4):
                    so = sb * 128
                    ps = pproj.tile([128, W65], fp32, tag="proj")
                    for kc in range(NKC):
                        nc.tensor.matmul(
                            ps[:],
                            xv_s[:, kc * S + so: kc * S + so + 128],
                            wv_s[:, kc * W65:(kc + 1) * W65],
                            start=(kc == 0), stop=(kc == NKC - 1))
                        yield None
                    nc.vector.tensor_add(vh_s[:, sb * W65:(sb + 1) * W65],
                                         ps[:], vb_s[:])
                    yield None
                yield f"v{c}"

            # deferred PV for an already-scored (sc, p): consumes the saved
            # exp tiles via the pproj pool (rows 0:65 of a [128, QC] tile)
            # so it never contends with the inline ppv pool.
            saved: dict = {}

            def pv_def(sc, p):
                n_kb = 4 * sc + 4 if mode == "causal" else N_KB
                pv = [pproj.tile([128, QC], fp32, tag="proj",
                                 name=f"dpv{b2}") for b2 in range(2)]
                for kb in range(n_kb):
                    et, t = saved[(sc, p)][kb]
                    for b2 in range(2):
                        h = 2 * p + b2
                        nc.tensor.matmul(
                            pv[b2][0:65, t:],
                            vh_s[:, kb * W65 + h * 65: kb * W65 + h * 65 + 65],
                            et[:, b2 * QC + t:(b2 + 1) * QC],
                            start=(kb == 0), stop=(kb == n_kb - 1))
                    yield None
                cs = slice(sc * QC, (sc + 1) * QC)
                for b2 in range(2):
                    h = 2 * p + b2
                    ot = oout.tile([65, QC], bf, tag="o")
                    nc.vector.tensor_copy(ot[:], pv[b2][0:65, :])
                    eng = nc.gpsimd if b2 == 0 else nc.sync
                    eng.dma_start(out[h, :, cs], ot[:])
                    yield None
                del saved[(sc, p)]
                yield f"pv{sc}{p}"

            class Filler:
                def __init__(self, *gens, preseen=()):
                    def chain():
                        for g in gens:
                            yield from g
                    self.g = chain()
                    self.seen = set(preseen)
                    self.done = False

                def pull(self, n=1):
                    for _ in range(n):
                        try:
                            lbl = next(self.g)
                        except StopIteration:
                            self.done = True
                            return
                        if lbl:
                            self.seen.add(lbl)

                def drain_until(self, lbl):
                    while lbl not in self.seen and not self.done:
                        self.pull()

                def drain(self):
                    while not self.done:
                        self.pull()

            def score_exp(sc, p, kb):
                cs = slice(sc * QC, (sc + 1) * QC)
                j = kb - 4 * sc if mode == "causal" else -1
                t = 128 * j if j > 0 else 0  # trimmed leading cols
                st = pscore.tile([128, 2 * QC], fp32, tag="s")
                for b2 in range(2):
                    nc.tensor.matmul(
                        st[:, b2 * QC + t:(b2 + 1) * QC],
                        kt_s[p][b2 * 64:(b2 + 1) * 64,
                                kb * 128:(kb + 1) * 128],
                        qt_s[p][b2 * 64:(b2 + 1) * 64,
                                sc * QC + t:(sc + 1) * QC],
                        start=True, stop=True)
                et = eload.tile([128, 2 * QC], bf, tag="e")
                if t > 0:
                    st3 = st[:].rearrange("p (h n) -> p h n", h=2)
                    et3 = et[:].rearrange("p (h n) -> p h n", h=2)
                    nc.scalar.activation(et3[:, :, t:], st3[:, :, t:],
                                         AF.Exp)
                else:
                    if mode == "general":
                        mt = mload.tile([128, QC], bf, tag="mt")
                        nc.sync.dma_start(
                            mt[:], amaskT[kb * 128:(kb + 1) * 128, cs])
                        for b2 in range(2):
                            nc.vector.tensor_add(
                                st[:, b2 * QC:(b2 + 1) * QC],
                                st[:, b2 * QC:(b2 + 1) * QC], mt[:])
                    nc.scalar.activation(et[:], st[:], AF.Exp)
                if j >= 0:
                    for b2 in range(2):
                        nc.vector.tensor_mul(
                            et[:, b2 * QC + t: b2 * QC + t + 128],
                            et[:, b2 * QC + t: b2 * QC + t + 128],
                            cmw_s[:])
                return et, t

            def flat_attention(fil, rates, defer_scs):
                """Score+exp stream per (sc, p, kb). PV for sc in defer_scs
                is saved for the pv_def filler generators; otherwise PV is
                emitted inline with a 2-pair lag (the ACT engine never waits
                for a phase turnover). Filler is pulled after every
                score_exp at rates[sc][p]."""
                pvs = {}
                pending = []

                def flush():
                    for f in pending:
                        f()
                    pending.clear()

                def pv_actions(sc, p, kb2, n_kb, pairs):
                    def emit():
                        fil.drain_until(f"v{sc}")
                        key = (sc, p)
                        if key not in pvs:
                            pvs[key] = [ppv.tile([65, QC], fp32, tag="pv",
                                                 name=f"pv{b2}")
                                        for b2 in range(2)]
                        pv = pvs[key]
                        for i, (et, t) in enumerate(pairs):
                            kb = 2 * kb2 + i
                            for b2 in range(2):
                                h = 2 * p + b2
                                nc.tensor.matmul(
                                    pv[b2][:, t:],
                                    vh_s[:, kb * W65 + h * 65:
                                            kb * W65 + h * 65 + 65],
                                    et[:, b2 * QC + t:(b2 + 1) * QC],
                                    start=(kb == 0), stop=(kb == n_kb - 1))
                        if kb2 == n_kb // 2 - 1:
                            cs = slice(sc * QC, (sc + 1) * QC)
                            for b2 in range(2):
                                h = 2 * p + b2
                                ot = oout.tile([65, QC], bf, tag="o")
                                nc.vector.tensor_copy(ot[:], pv[b2][:])
                                eng = nc.gpsimd if b2 == 0 else nc.sync
                                eng.dma_start(out[h, :, cs], ot[:])
                            del pvs[key]
                    return emit

                for sc in range(N_QC):
                    n_kb = 4 * sc + 4 if mode == "causal" else N_KB
                    for p in range(2):
                        if p == 0:
                            fil.drain_until(f"kq{sc}m0")
                            if sc == 2 and 0 in defer_scs:
                                fil.drain_until("pv01")
                        else:
                            fil.drain_until(f"kq{sc}")
                            if sc == 2 and 1 in defer_scs:
                                fil.drain_until("pv11")
                        for kb2 in range(n_kb // 2):
                            if sc == 3 and p == 0 and 2 in defer_scs:
                                if kb2 == 2:
                                    fil.drain_until("pv20")
                                elif kb2 == 4:
                                    fil.drain_until("pv21")
                            e0 = score_exp(sc, p, 2 * kb2)
                            fil.pull(rates[sc][p])
                            e1 = score_exp(sc, p, 2 * kb2 + 1)
                            fil.pull(rates[sc][p])
                            if sc in defer_scs:
                                saved.setdefault((sc, p), []).extend([e0, e1])
                            else:
                                while len(pending) >= 2:
                                    pending.pop(0)()
                                pending.append(
                                    pv_actions(sc, p, kb2, n_kb, [e0, e1]))
                flush()

            if mode == "causal":
                fil = Filler(proj_kq_chunk(0), proj_kq_chunk(1), proj_v(0),
                             pv_def(0, 0), pv_def(0, 1),
                             proj_kq_chunk(2), proj_v(2), proj_v(1),
                             pv_def(1, 0), pv_def(1, 1),
                             proj_kq_chunk(3), proj_v(3),
                             pv_def(2, 0), pv_def(2, 1))
                flat_attention(fil, rates=[[5, 5], [6, 6], [7, 2], [4, 3]],
                               defer_scs={0, 1, 2})
                fil.drain()
            else:
                fil = Filler(proj_kq_chunk(0), proj_v(0), proj_v(1),
                             proj_kq_chunk(1), proj_v(2), proj_v(3),
                             proj_kq_chunk(2), proj_kq_chunk(3))
                fil.drain()
                nofil = Filler(preseen={f"kq{c}{s}" for c in range(4)
                                        for s in ("", "m0")}
                               | {f"v{c}" for c in range(4)})
                flat_attention(nofil, rates=[[0, 0]] * 4, defer_scs=set())

    nc.compile()
    return nc


def _get_nc(mode: str):
    if mode not in _cache:
        _cache[mode] = _build(mode)
    return _cache[mode]


def kernel(q, k, v, mask, Wq, bq, Wk, bk, Wv, bv):
    q = np.asarray(q, np.float32)
    k = np.asarray(k, np.float32)
    v = np.asarray(v, np.float32)
    Wq = np.asarray(Wq, np.float32)
    Wk = np.asarray(Wk, np.float32)
    Wv = np.asarray(Wv, np.float32)
    bq = np.asarray(bq, np.float32)
    bk = np.asarray(bk, np.float32)
    bv = np.asarray(bv, np.float32)
    m2 = np.asarray(mask)[0, 0]

    causal = bool(np.array_equal(m2 != 0, np.tril(np.ones((S, S), bool))))
    if causal:
        mode = "causal"
    elif np.all(m2 != 0):
        mode = "none"
    else:
        mode = "general"

    from concourse.bass_utils import run_bass_kernel_spmd

    nc = _get_nc(mode)

    def packx(xb):
        # [S, D] -> [128, (kc, s)] SBUF image
        return np.ascontiguousarray(
            xb.T.reshape(NKC, 128, S).transpose(1, 0, 2).reshape(128, NKC * S)
        ).astype(bf16)

    in_maps = []
    for c in range(N_CORES):
        b, g = divmod(c, HPC)
        gsl = slice(g * GD, (g + 1) * GD)
        # V weights: per head 65 cols (64 data + zero col for the ones dim);
        # the ones + bias come from the broadcast add tile vb.
        wv65 = np.zeros((D, W65), np.float32)
        vbrow = np.zeros((1, W65), np.float32)
        for h in range(HPC):
            wv65[:, h * 65:h * 65 + 64] = Wv[g * GD + h * DK:
                                             g * GD + h * DK + DK, :].T
            vbrow[0, h * 65:h * 65 + 64] = bv[g * GD + h * DK:
                                              g * GD + h * DK + DK]
            vbrow[0, h * 65 + 64] = 1.0

        def packw(wt):
            n = wt.shape[1]
            return np.ascontiguousarray(
                wt.reshape(NKC, 128, n).transpose(1, 0, 2).reshape(128, NKC * n)
            ).astype(bf16)

        def packw_mmajor(wt):
            # [D, 256] -> [128, (m, kc, 128)] so each m-half is contiguous
            return np.ascontiguousarray(
                wt.reshape(NKC, 128, 2, 128).transpose(1, 2, 0, 3)
                  .reshape(128, NKC * GD)).astype(bf16)

        im = {
            "xq": packx(q[b]),
            "xk": packx(k[b]),
            "xv": packx(v[b]),
            "wq": packw_mmajor(Wq[gsl, :].T / 8.0),
            "wk": packw_mmajor(Wk[gsl, :].T),
            "wv": packw(wv65),
            "vb": np.broadcast_to(vbrow, (128, W65)).astype(bf16).copy(),
            "bqk": np.stack([bq[gsl][:128] / 8.0, bq[gsl][128:] / 8.0,
                             bk[gsl][:128], bk[gsl][128:]], 1)
                     .astype(np.float32).copy(),
        }
        if mode == "causal":
            r = np.arange(128)[:, None]
            cc = np.arange(128)[None, :]
            im["cmw"] = np.where(r <= cc, 1.0, 0.0).astype(bf16)
        elif mode == "general":
            add = np.where(m2 == 0, -1e9, 0.0).astype(np.float32)
            im["amaskT"] = add.T.astype(bf16).copy()
        in_maps.append(im)

    global _last_in_maps
    _last_in_maps = in_maps
    res = run_bass_kernel_spmd(nc, in_maps, core_ids=list(range(N_CORES)))

    outf = np.empty((B, S, D), np.float32)
    for c in range(N_CORES):
        b, g = divmod(c, HPC)
        o = np.asarray(res.results[c]["out"], np.float32)  # [HPC, 65, S]
        num = o[:, :64, :]         # [HPC, 64, S]
        den = o[:, 64:65, :]       # [HPC, 1, S]
        oh = num / den             # [HPC, 64, S]
        outf[b, :, g * GD:(g + 1) * GD] = (
            oh.transpose(2, 0, 1).reshape(S, GD))
    return outf
